# revision 13
# baseline (speedup 1.0000x reference)
"""DLRM DotInteraction kernel for 8x Trainium2 NeuronCores.

Full input x: [16384, 32, 64] f32. Per batch b: G = x_b @ x_b^T [32, 32];
output = strict lower triangle of G, row-major -> [16384, 496] f32.

Sharding: pure data parallel, 2048 batches per core.

v6 design (shared LDWEIGHTS + column-tiled matmuls, K=128 zero-padded):
  - X buffers [128, 2, 64, 32] fp16: region 0 holds 64 "A" batches' x^T
    on partitions 0:64 (d on partitions) with ZEROS on 64:128; region 1
    holds 64 "B" batches on 64:128 with zeros on 0:64. Zero cells are
    memset once on NB persistent buffers; per-load DMAs (two
    complementary 64-partition transfers hitting disjoint SDMA engine
    sets -> full aggregate bandwidth) overwrite only data cells.
  - per quad (4 consecutive same-region cells, contiguous 128 cols):
    ONE explicit LDWEIGHTS loads all 4 cells into the PE array, then
    four non-self-loading (ldweights=False) N=32 matmuls, one per
    32-column subarray (tile_position (0, 32j), tile_size (128, 32)).
    Each matmul computes cell_j^T @ cell_j = G_j in isolation (column
    tiling ignores the other cells; the K=128 zero half kills the
    bottom/top contribution). The 4 matmuls run concurrently in
    disjoint column groups. One 26ns-class weight load is amortized
    over 4 batches; the per-matmul fixed cost (~26-35ns empirical) is
    the dominant PE tax otherwise. Everything at tile_position row 0:
    dodges the HW erratum where mixing row offsets hangs the exec unit.
  - PSUM: out [32, 32] at partition block 32j -> four grams stack a
    fully-useful [128, 32] slot; 16 slots per bank = 64 batches.
  - extraction: per bank two full-width [128, 8, 32] f32->fp16 copies
    (Vector + Scalar engines), zero garbage -> ~420ns/bank, far below
    the PE rate, so no backpressure and the HAM clock can ramp.
  - dump: per 256 batches one [128, 4KB] fp16 DMA (sync engine, so the
    copy engines never stall on dump semaphores); triangle on host.
"""

import numpy as np

import concourse.bass as bass
import concourse.tile as tile
from concourse import mybir
from concourse.tile import add_dep_helper
from concourse.bass_utils import run_bass_kernel_spmd

N_CORES = 8
B_FULL = 16384
B = B_FULL // N_CORES  # 2048 batches per core
F = 32
D = 64
NPAIR = F * (F - 1) // 2  # 496

FP16 = mybir.dt.float16
FP32 = mybir.dt.float32

LOAD_B = 128            # batches per load (two half-width DMAs)
N_LOADS = B // LOAD_B   # 16
NB = 3                  # persistent X buffers
BANK_B = 64             # batches per PSUM bank (16 slots x 4 blocks)
GROUP_B = 256           # batches per staging/dump group
BANKS_PER_GROUP = 4
N_GROUPS = 8
QUADS_PER_LOAD = 32


def split_multiwait_insts(nc):
    """walrus in this env allows only one sem wait per instruction; the tile
    tail drain carries several. Hoist extras onto preceding single-wait NoOps."""
    for func in nc.m.functions:
        for blk in func.blocks:
            insts = list(blk.instructions)
            changed = False
            new_list = []
            for inst in insts:
                si = inst.sync_info
                if si is not None and len(si.on_wait) > 1:
                    waits = list(si.on_wait)
                    for k, w in enumerate(waits[1:]):
                        new_list.append(
                            mybir.InstNoOp(
                                name=f"{inst.name}-wsplit{k}",
                                engine=inst.engine,
                                sync_info=mybir.SyncInfo(on_wait=[w], on_update=[]),
                                bass_nofuse=True,
                            )
                        )
                    inst.sync_info = mybir.SyncInfo(
                        on_wait=[waits[0]], on_update=list(si.on_update)
                    )
                    changed = True
                new_list.append(inst)
            if changed:
                blk.instructions = new_list


def host_prep(xc):
    """[2048, 32, 64] f32 -> [128, 16, 2048] fp16.

    Row 64*par + d, col (t, p, f): batch 128*t + 64*par + p."""
    xv = xc.reshape(N_LOADS, 2, 64, F, D)      # [t, par, p, f, d]
    xq = xv.transpose(1, 4, 0, 2, 3)           # [par, d, t, p, f]
    return np.ascontiguousarray(
        xq.reshape(128, N_LOADS, 2048).astype(np.float16)
    )


def build_program():
    nc = bass.Bass()
    xq = nc.declare_dram_parameter("xq", [128, N_LOADS, 2048], FP16,
                                   isOutput=False)
    dump = nc.declare_dram_parameter(
        "dump", [N_GROUPS, 128, GROUP_B * 8], FP16, isOutput=True
    )

    with tile.TileContext(nc) as tc:
        with (
            tc.tile_pool(name="xin", bufs=NB) as xpool,
            tc.tile_pool(name="stage", bufs=2) as spool,
            tc.tile_pool(name="psum_g", bufs=6, space="PSUM") as psumG,
        ):
            # persistent X buffers; zero cells written once (vector +
            # gpsimd in parallel, interleaved with the first loads)
            Xs = [xpool.tile([128, 2, 64, F], FP16, name=f"X{b}")
                  for b in range(NB)]

            def load(t):
                X = Xs[t % NB]
                nc.sync.dma_start(X[0:64, 0], xq[0:64, t, :])
                nc.sync.dma_start(X[64:128, 1], xq[64:128, t, :])

            for t0 in range(NB):
                nc.vector.memset(Xs[t0][64:128, 0], 0.0)
                nc.gpsimd.memset(Xs[t0][0:64, 1], 0.0)
                load(t0)

            for grp in range(N_GROUPS):
                S = spool.tile([128, BANKS_PER_GROUP, 16, F], FP16)
                s_copies = []
                for g4 in range(BANKS_PER_GROUP):
                    bank = grp * BANKS_PER_GROUP + g4
                    pG = psumG.tile([128, 16, F], FP32)  # 1 bank, 16 slots
                    for sl in range(16):
                        Q = bank * 16 + sl
                        t, qin = Q // QUADS_PER_LOAD, Q % QUADS_PER_LOAD
                        r, p0 = qin % 2, 4 * (qin // 2)
                        X = Xs[t % NB]
                        nc.tensor.ldweights(X[:, r, p0 : p0 + 4, :],
                                            tile_position=(0, 0))
                        for j in range(4):
                            cell = X[:, r, p0 + j, :]    # [128, 32]
                            mm = nc.tensor.matmul(
                                pG[32 * j : 32 * j + 32, sl],
                                lhsT=cell, rhs=cell,
                                start=True, stop=True,
                                tile_position=(0, 32 * j),
                            )
                            mm.ins.ldweights = False
                        # prefetch into the buffer this load just freed;
                        # issued after the load's last matmul so the WAR
                        # dep orders the DMA behind those reads
                        if qin == QUADS_PER_LOAD - 1 and t + NB < N_LOADS:
                            load(t + NB)
                    for half in range(2):
                        src = pG[:, 8 * half : 8 * half + 8, :]
                        dst = S[:, g4, 8 * half : 8 * half + 8, :]
                        if half == 0:
                            cp = nc.vector.tensor_copy(dst, src)
                        else:
                            cp = nc.scalar.copy(dst, src)
                        s_copies.append(cp.ins)
                g = nc.sync.dma_start(dump[grp], S[:])
                for cp_inst in s_copies:
                    add_dep_helper(g.ins, cp_inst, sync=True)

    split_multiwait_insts(nc)
    return nc


_CACHED = None


def _get_program():
    global _CACHED
    if _CACHED is None:
        _CACHED = build_program()
    return _CACHED


_TRIL_ROWS, _TRIL_COLS = np.tril_indices(F, k=-1)


def _batch_map():
    """batch index for each (grp, j, g4, sl) dump coordinate.

    dump[grp, 128p, 2048]: partition p = 32*j + f; cols (g4, sl, c).
    Quad Q = bank*16 + sl held batches 128*t + 64*r + p0 + j."""
    grp, j, g4, sl = np.meshgrid(
        np.arange(N_GROUPS), np.arange(4), np.arange(BANKS_PER_GROUP),
        np.arange(16), indexing="ij",
    )
    bank = grp * BANKS_PER_GROUP + g4
    Q = bank * 16 + sl
    t, qin = Q // QUADS_PER_LOAD, Q % QUADS_PER_LOAD
    r, p0 = qin % 2, 4 * (qin // 2)
    return (LOAD_B * t + 64 * r + p0 + j).ravel()


_BATCH_MAP = _batch_map()


def _unpack_dump(d):
    """[8, 128, 2048] fp16 dump -> [2048, 496] f32 triangle rows."""
    d6 = d.reshape(N_GROUPS, 4, F, BANKS_PER_GROUP, 16, F)  # grp,j,f,g4,sl,c
    d6 = d6.transpose(0, 1, 3, 4, 2, 5)                     # grp,j,g4,sl,f,c
    G = np.empty((B, F, F), dtype=np.float32)
    G[_BATCH_MAP] = d6.reshape(-1, F, F).astype(np.float32)
    return G[:, _TRIL_ROWS, _TRIL_COLS]


def kernel(**inputs) -> np.ndarray:
    x = np.asarray(inputs["x"], dtype=np.float32)
    assert x.shape == (B_FULL, F, D), x.shape
    nc = _get_program()
    in_maps = [host_prep(x[i * B : (i + 1) * B]) for i in range(N_CORES)]
    res = run_bass_kernel_spmd(
        nc, [{"xq": m} for m in in_maps], list(range(N_CORES))
    )
    return np.concatenate(
        [_unpack_dump(res.results[i]["dump"]) for i in range(N_CORES)], axis=0
    ).astype(np.float32)


# revision 15
# speedup vs baseline: 1.1616x; 1.1616x over previous
"""DLRM DotInteraction kernel for 8x Trainium2 NeuronCores.

Full input x: [16384, 32, 64] f32. Per batch b: G = x_b @ x_b^T [32, 32];
output = strict lower triangle of G, row-major -> [16384, 496] f32.

Sharding: pure data parallel, 2048 batches per core.

v6 design (shared LDWEIGHTS + column-tiled matmuls, K=128 zero-padded):
  - X buffers [128, 2, 64, 32] fp16: region 0 holds 64 "A" batches' x^T
    on partitions 0:64 (d on partitions) with ZEROS on 64:128; region 1
    holds 64 "B" batches on 64:128 with zeros on 0:64. Zero cells are
    memset once on NB persistent buffers; per-load DMAs (two
    complementary 64-partition transfers hitting disjoint SDMA engine
    sets -> full aggregate bandwidth) overwrite only data cells.
  - per quad (4 consecutive same-region cells, contiguous 128 cols):
    ONE explicit LDWEIGHTS loads all 4 cells into the PE array, then
    four non-self-loading (ldweights=False) N=32 matmuls, one per
    32-column subarray (tile_position (0, 32j), tile_size (128, 32)).
    Each matmul computes cell_j^T @ cell_j = G_j in isolation (column
    tiling ignores the other cells; the K=128 zero half kills the
    bottom/top contribution). The 4 matmuls run concurrently in
    disjoint column groups. One 26ns-class weight load is amortized
    over 4 batches; the per-matmul fixed cost (~26-35ns empirical) is
    the dominant PE tax otherwise. Everything at tile_position row 0:
    dodges the HW erratum where mixing row offsets hangs the exec unit.
  - PSUM: out [32, 32] at partition block 32j -> four grams stack a
    fully-useful [128, 32] slot; 16 slots per bank = 64 batches.
  - extraction: per bank two full-width [128, 8, 32] f32->fp16 copies
    (Vector + Scalar engines), zero garbage -> ~420ns/bank, far below
    the PE rate, so no backpressure and the HAM clock can ramp.
  - dump: per 256 batches one [128, 4KB] fp16 DMA (sync engine, so the
    copy engines never stall on dump semaphores); triangle on host.
"""

import numpy as np

import concourse.bass as bass
import concourse.tile as tile
from concourse import mybir
from concourse.tile import add_dep_helper
from concourse.bass_utils import run_bass_kernel_spmd

N_CORES = 8
B_FULL = 16384
B = B_FULL // N_CORES  # 2048 batches per core
F = 32
D = 64
NPAIR = F * (F - 1) // 2  # 496

FP16 = mybir.dt.float16
FP32 = mybir.dt.float32

LOAD_B = 128            # batches per load (two half-width DMAs)
N_LOADS = B // LOAD_B   # 16
NB = 3                  # persistent X buffers
BANK_B = 64             # batches per PSUM bank (16 slots x 4 blocks)
GROUP_B = 256           # batches per staging/dump group
BANKS_PER_GROUP = 4
N_GROUPS = 8
QUADS_PER_LOAD = 32


def split_multiwait_insts(nc):
    """walrus in this env allows only one sem wait per instruction; the tile
    tail drain carries several. Hoist extras onto preceding single-wait NoOps."""
    for func in nc.m.functions:
        for blk in func.blocks:
            insts = list(blk.instructions)
            changed = False
            new_list = []
            for inst in insts:
                si = inst.sync_info
                if si is not None and len(si.on_wait) > 1:
                    waits = list(si.on_wait)
                    for k, w in enumerate(waits[1:]):
                        new_list.append(
                            mybir.InstNoOp(
                                name=f"{inst.name}-wsplit{k}",
                                engine=inst.engine,
                                sync_info=mybir.SyncInfo(on_wait=[w], on_update=[]),
                                bass_nofuse=True,
                            )
                        )
                    inst.sync_info = mybir.SyncInfo(
                        on_wait=[waits[0]], on_update=list(si.on_update)
                    )
                    changed = True
                new_list.append(inst)
            if changed:
                blk.instructions = new_list


def host_prep(xc):
    """[2048, 32, 64] f32 -> [128, 16, 2048] fp16.

    Row 64*par + d, col (t, p, f): batch 128*t + 64*par + p."""
    xv = xc.reshape(N_LOADS, 2, 64, F, D)      # [t, par, p, f, d]
    xq = xv.transpose(1, 4, 0, 2, 3)           # [par, d, t, p, f]
    return np.ascontiguousarray(
        xq.reshape(128, N_LOADS, 2048).astype(np.float16)
    )


def build_program():
    nc = bass.Bass()
    xq = nc.declare_dram_parameter("xq", [128, N_LOADS, 2048], FP16,
                                   isOutput=False)
    dump = nc.declare_dram_parameter(
        "dump", [N_GROUPS, 128, GROUP_B * 8], FP16, isOutput=True
    )

    with tile.TileContext(nc) as tc:
        with (
            tc.tile_pool(name="xin", bufs=NB) as xpool,
            tc.tile_pool(name="stage", bufs=2) as spool,
            tc.tile_pool(name="psum_g", bufs=2, space="PSUM") as psumG,
        ):
            # persistent X buffers; zero cells written once (vector +
            # gpsimd in parallel, interleaved with the first loads)
            Xs = [xpool.tile([128, 2, 64, F], FP16, name=f"X{b}")
                  for b in range(NB)]

            def load(t):
                X = Xs[t % NB]
                nc.sync.dma_start(X[0:64, 0], xq[0:64, t, :])
                nc.sync.dma_start(X[64:128, 1], xq[64:128, t, :])

            for t0 in range(NB):
                nc.vector.memset(Xs[t0][64:128, 0], 0.0)
                nc.gpsimd.memset(Xs[t0][0:64, 1], 0.0)
                load(t0)

            for grp in range(N_GROUPS):
                S = spool.tile([128, BANKS_PER_GROUP, 16, F], FP16)
                s_copies = []
                for g4 in range(BANKS_PER_GROUP):
                    bank = grp * BANKS_PER_GROUP + g4
                    # 4 PSUM banks, 16 quad slots = 64 batches
                    pG = psumG.tile([128, 16, 4, F], FP32)
                    for sl in range(16):
                        Q = bank * 16 + sl
                        t, qin = Q // QUADS_PER_LOAD, Q % QUADS_PER_LOAD
                        r, p0 = qin % 2, 4 * (qin // 2)
                        X = Xs[t % NB]
                        quad = X[:, r, p0 : p0 + 4, :]   # [128, 4, 32]
                        nc.tensor.matmul(pG[:, sl], lhsT=quad, rhs=quad,
                                         start=True, stop=True)
                        # prefetch into the buffer this load just freed;
                        # issued after the load's last matmul so the WAR
                        # dep orders the DMA behind those reads
                        if qin == QUADS_PER_LOAD - 1 and t + NB < N_LOADS:
                            load(t + NB)
                    for j in range(4):
                        src = pG[32 * j : 32 * j + 32, :, j, :]
                        dst = S[32 * j : 32 * j + 32, g4, :, :]
                        if j < 2:
                            cp = nc.vector.tensor_copy(dst, src)
                        else:
                            cp = nc.scalar.copy(dst, src)
                        s_copies.append(cp.ins)
                g = nc.sync.dma_start(dump[grp], S[:])
                for cp_inst in s_copies:
                    add_dep_helper(g.ins, cp_inst, sync=True)

    split_multiwait_insts(nc)
    return nc


_CACHED = None


def _get_program():
    global _CACHED
    if _CACHED is None:
        _CACHED = build_program()
    return _CACHED


_TRIL_ROWS, _TRIL_COLS = np.tril_indices(F, k=-1)


def _batch_map():
    """batch index for each (grp, j, g4, sl) dump coordinate.

    dump[grp, 128p, 2048]: partition p = 32*j + f; cols (g4, sl, c).
    Quad Q = bank*16 + sl held batches 128*t + 64*r + p0 + j."""
    grp, j, g4, sl = np.meshgrid(
        np.arange(N_GROUPS), np.arange(4), np.arange(BANKS_PER_GROUP),
        np.arange(16), indexing="ij",
    )
    bank = grp * BANKS_PER_GROUP + g4
    Q = bank * 16 + sl
    t, qin = Q // QUADS_PER_LOAD, Q % QUADS_PER_LOAD
    r, p0 = qin % 2, 4 * (qin // 2)
    return (LOAD_B * t + 64 * r + p0 + j).ravel()


_BATCH_MAP = _batch_map()


def _unpack_dump(d):
    """[8, 128, 2048] fp16 dump -> [2048, 496] f32 triangle rows."""
    d6 = d.reshape(N_GROUPS, 4, F, BANKS_PER_GROUP, 16, F)  # grp,j,f,g4,sl,c
    d6 = d6.transpose(0, 1, 3, 4, 2, 5)                     # grp,j,g4,sl,f,c
    G = np.empty((B, F, F), dtype=np.float32)
    G[_BATCH_MAP] = d6.reshape(-1, F, F).astype(np.float32)
    return G[:, _TRIL_ROWS, _TRIL_COLS]


def kernel(**inputs) -> np.ndarray:
    x = np.asarray(inputs["x"], dtype=np.float32)
    assert x.shape == (B_FULL, F, D), x.shape
    nc = _get_program()
    in_maps = [host_prep(x[i * B : (i + 1) * B]) for i in range(N_CORES)]
    res = run_bass_kernel_spmd(
        nc, [{"xq": m} for m in in_maps], list(range(N_CORES))
    )
    return np.concatenate(
        [_unpack_dump(res.results[i]["dump"]) for i in range(N_CORES)], axis=0
    ).astype(np.float32)


# revision 22
# speedup vs baseline: 1.1991x; 1.0323x over previous
"""DLRM DotInteraction kernel for 8x Trainium2 NeuronCores.

Full input x: [16384, 32, 64] f32. Per batch b: G = x_b @ x_b^T [32, 32];
output = strict lower triangle of G, row-major -> [16384, 496] f32.

Sharding: pure data parallel, 2048 batches per core.

v5 design (K=128 zero-padded cells, rotating PSUM column blocks):
  - X buffers [128, 2, 64, 32] fp16: region 0 holds 64 "A" batches' x^T
    on partitions 0:64 (d on partitions) with ZEROS on 64:128; region 1
    holds 64 "B" batches on 64:128 with zeros on 0:64. Zero cells are
    memset once on NB persistent buffers (~0.6us each); per-load DMAs
    (two complementary 64-partition transfers hitting disjoint SDMA
    engine sets -> full aggregate bandwidth) overwrite only data cells.
  - per batch: ONE matmul, lhsT = rhs = its cell [128, 32] (contiguous,
    single free dim). K=128 with the zero half contributing nothing, so
    every matmul runs at tile_position row 0 — dodges the HW erratum
    where mixing row offsets 0/64 hangs the exec unit. Out [32, 32]
    goes to PSUM partition block 32*j with j rotating 0..3, so four
    batches tile a fully-useful [128, 32] slot and consecutive
    LDWEIGHTS/MATMUL hit disjoint PE column groups (they overlap).
  - extraction: per PSUM bank (16 slots = 64 batches) two full-width
    [128, 8, 32] f32->fp16 copies (Vector + Scalar engines), zero
    garbage, ~2x faster than the PE stream -> no backpressure, PE duty
    stays high and the HAM clock ramps to 2.4 GHz.
  - dump: per 256 batches one [128, 4KB] fp16 DMA of the full grams;
    the strict-lower-triangle gather happens on host.
"""

import numpy as np

import concourse.bass as bass
import concourse.tile as tile
from concourse import mybir
from concourse.tile import add_dep_helper
from concourse.bass_utils import run_bass_kernel_spmd

N_CORES = 8
B_FULL = 16384
B = B_FULL // N_CORES  # 2048 batches per core
F = 32
D = 64
NPAIR = F * (F - 1) // 2  # 496

FP16 = mybir.dt.float16
FP32 = mybir.dt.float32

LOAD_B = 128            # batches per load (two half-width DMAs)
N_LOADS = B // LOAD_B   # 16
NB = 3                  # persistent X buffers
BANK_B = 64             # batches per PSUM bank (16 slots x 4 blocks / 1)
GROUP_B = 256           # batches per staging/dump group
BANKS_PER_GROUP = 4
N_BANKS = B // BANK_B   # 32
N_GROUPS = 8


def split_multiwait_insts(nc):
    """walrus in this env allows only one sem wait per instruction; the tile
    tail drain carries several. Hoist extras onto preceding single-wait NoOps."""
    for func in nc.m.functions:
        for blk in func.blocks:
            insts = list(blk.instructions)
            changed = False
            new_list = []
            for inst in insts:
                si = inst.sync_info
                if si is not None and len(si.on_wait) > 1:
                    waits = list(si.on_wait)
                    for k, w in enumerate(waits[1:]):
                        new_list.append(
                            mybir.InstNoOp(
                                name=f"{inst.name}-wsplit{k}",
                                engine=inst.engine,
                                sync_info=mybir.SyncInfo(on_wait=[w], on_update=[]),
                                bass_nofuse=True,
                            )
                        )
                    inst.sync_info = mybir.SyncInfo(
                        on_wait=[waits[0]], on_update=list(si.on_update)
                    )
                    changed = True
                new_list.append(inst)
            if changed:
                blk.instructions = new_list


def host_prep(xc):
    """[2048, 32, 64] f32 -> {"xq": [128, 16, 2048], "xz": [128, NB, 4096]}.

    xq row 64*par + d, col (t, p, f): batch 128*t + 64*par + p.
    xz holds full zero-padded X-buffer images for the first NB loads
    (region 0: data on partitions 0:64, zeros below; region 1 reversed)
    so no on-chip memsets are needed before compute starts."""
    xv = xc.reshape(N_LOADS, 2, 64, F, D)      # [t, par, p, f, d]
    xq = np.ascontiguousarray(
        xv.transpose(1, 4, 0, 2, 3).reshape(128, N_LOADS, 2048)
        .astype(np.float16)
    )
    xz = np.zeros((128, NB, 2, 2048), dtype=np.float16)
    for t in range(NB):
        xz[0:64, t, 0, :] = xq[0:64, t, :]
        xz[64:128, t, 1, :] = xq[64:128, t, :]
    return {"xq": xq, "xz": xz.reshape(128, NB, 4096)}


def build_program():
    nc = bass.Bass()
    xq = nc.declare_dram_parameter("xq", [128, N_LOADS, 2048], FP16,
                                   isOutput=False)
    xz = nc.declare_dram_parameter("xz", [128, NB, 4096], FP16,
                                   isOutput=False)
    dump = nc.declare_dram_parameter(
        "dump", [N_GROUPS, 128, GROUP_B * 8], FP16, isOutput=True
    )

    with tile.TileContext(nc) as tc:
        with (
            tc.tile_pool(name="xin", bufs=NB) as xpool,
            tc.tile_pool(name="stage", bufs=3) as spool,
            tc.tile_pool(name="psum_g", bufs=6, space="PSUM") as psumG,
        ):
            # persistent X buffers; the first NB loads bring full
            # zero-padded images, later loads overwrite only data cells
            Xs = [xpool.tile([128, 2, 64, F], FP16, name=f"X{b}")
                  for b in range(NB)]

            def load(t):
                X = Xs[t % NB]
                if t < NB:
                    nc.sync.dma_start(X[:], xz[:, t, :])
                else:
                    nc.sync.dma_start(X[0:64, 0], xq[0:64, t, :])
                    nc.sync.dma_start(X[64:128, 1], xq[64:128, t, :])

            for t0 in range(NB):
                load(t0)
            for grp in range(N_GROUPS):
                S = spool.tile([128, BANKS_PER_GROUP, 16, F], FP16)
                s_copies = []
                for g4 in range(BANKS_PER_GROUP):
                    bank = grp * BANKS_PER_GROUP + g4
                    t, p0 = bank // 2, (bank % 2) * 32
                    X = Xs[t % NB]
                    pG = psumG.tile([128, 16, F], FP32)  # 1 bank, 16 slots
                    for i in range(BANK_B):
                        sl, j = i // 4, i % 4
                        r, p = i % 2, p0 + i // 2
                        cell = X[:, r, p, :]             # [128, 32]
                        nc.tensor.matmul(
                            pG[32 * j : 32 * j + 32, sl], lhsT=cell, rhs=cell,
                            start=True, stop=True, tile_position=(0, 32 * j),
                        )
                    # prefetch into the buffer this load just freed; issued
                    # after the load's last matmul so the WAR dep orders the
                    # DMA behind those reads, not ahead of them
                    if bank % 2 == 1 and t + NB < N_LOADS:
                        load(t + NB)
                    for half in range(2):
                        src = pG[:, 8 * half : 8 * half + 8, :]
                        dst = S[:, g4, 8 * half : 8 * half + 8, :]
                        if half == 0:
                            cp = nc.vector.tensor_copy(dst, src)
                        else:
                            cp = nc.scalar.copy(dst, src)
                        s_copies.append(cp.ins)
                g = nc.gpsimd.dma_start(dump[grp], S[:])
                for cp_inst in s_copies:
                    add_dep_helper(g.ins, cp_inst, sync=True)

    split_multiwait_insts(nc)
    return nc


_CACHED = None


def _get_program():
    global _CACHED
    if _CACHED is None:
        _CACHED = build_program()
    return _CACHED


_TRIL_ROWS, _TRIL_COLS = np.tril_indices(F, k=-1)


def _batch_map():
    """batch index for each (grp, j, g4, sl) dump coordinate.

    dump[grp, 128p, 2048]: partition p = 32*j + f; cols (g4, sl, c).
    Bank cell i = 4*sl + j held batch 128*t + 64*(i%2) + p0 + i//2."""
    grp, j, g4, sl = np.meshgrid(
        np.arange(N_GROUPS), np.arange(4), np.arange(BANKS_PER_GROUP),
        np.arange(16), indexing="ij",
    )
    bank = grp * BANKS_PER_GROUP + g4
    t, p0 = bank // 2, (bank % 2) * 32
    i = 4 * sl + j
    return (LOAD_B * t + 64 * (i % 2) + p0 + i // 2).ravel()


_BATCH_MAP = _batch_map()


def _unpack_dump(d):
    """[8, 128, 2048] fp16 dump -> [2048, 496] f32 triangle rows."""
    d6 = d.reshape(N_GROUPS, 4, F, BANKS_PER_GROUP, 16, F)  # grp,j,f,g4,sl,c
    d6 = d6.transpose(0, 1, 3, 4, 2, 5)                     # grp,j,g4,sl,f,c
    G = np.empty((B, F, F), dtype=np.float32)
    G[_BATCH_MAP] = d6.reshape(-1, F, F).astype(np.float32)
    return G[:, _TRIL_ROWS, _TRIL_COLS]


def kernel(**inputs) -> np.ndarray:
    x = np.asarray(inputs["x"], dtype=np.float32)
    assert x.shape == (B_FULL, F, D), x.shape
    nc = _get_program()
    in_maps = [host_prep(x[i * B : (i + 1) * B]) for i in range(N_CORES)]
    res = run_bass_kernel_spmd(nc, in_maps, list(range(N_CORES)))
    return np.concatenate(
        [_unpack_dump(res.results[i]["dump"]) for i in range(N_CORES)], axis=0
    ).astype(np.float32)
